# revision 23
# baseline (speedup 1.0000x reference)
"""Trainium2 Bass kernel for nn_Attention (B=4, S=2048, D=1024, H=16, hd=64, fp32).

Sharding (zero-communication): 8 cores; core c handles batch b=c//2 and
query-half qh=c%2. Each core computes K,V for its whole batch (all heads),
Q for its query half, attention for all 16 heads over its 1024 queries, and
the output projection for its 1024 rows. The per-core input x is permuted so
the core's query half comes first (softmax over keys is permutation
invariant, so K/V may use the permuted order as long as they agree).

Per-core pipeline (all matmuls bf16, accumulation fp32 in PSUM):
  A. xT[D,S] built via PE transposes of x tiles (bf16).
  B. KT[hd,S] / QT[hd,Sq] (heads stacked 2-per-128-partitions), V[S,hd]
     augmented with a ones column (gives the softmax denominator for free).
     Emission order K0,Q0 -> V -> K1..7,Q1..7 interleaved with attention so
     ScalarE exp work starts as early as possible.
  C. scoresT[k,q] via PE (two heads row-packed with tile_position), exp on
     ScalarE (no max subtraction: |scores/8| < ~3 by construction), attnV
     accumulates (P @ V)^T; the ones column produces l[q]; normalization via
     reciprocal + K=1 broadcast matmuls fused into the PSUM evacuation.
  D. y = outT^T @ W_proj + b_proj (bias via K=1 ones matmul).

One shared [128,1024] PSUM pool (3 slots) + a [65,512] accumulator pool
(2 slots) keeps all phases inside the 8 PSUM banks with fine-grained
slot-level WAR deps instead of phase barriers.
"""

import os
import sys

import numpy as np

B, S, D, H, HD = 4, 2048, 1024, 16, 64
QH = 1024  # queries per core
NC_ = 8

_cache = {}


def _build_nc():
    sys.path.insert(0, "/opt/trn_rl_repo")
    import concourse.bass as bass
    from concourse import bacc
    import concourse.mybir as mybir
    import concourse.tile as tile
    from concourse.masks import make_identity
    from contextlib import ExitStack

    F32 = mybir.dt.float32
    BF16 = mybir.dt.bfloat16
    MULT = mybir.AluOpType.mult
    Exp = mybir.ActivationFunctionType.Exp

    nc = bacc.Bacc()
    x_d = nc.declare_dram_parameter("xb", [S, D], F32, isOutput=False)
    wq_d = nc.declare_dram_parameter("wq", [D, D], F32, isOutput=False)
    wk_d = nc.declare_dram_parameter("wk", [D, D], F32, isOutput=False)
    wv_d = nc.declare_dram_parameter("wv", [D, D], F32, isOutput=False)
    wp_d = nc.declare_dram_parameter("wp", [D, D], F32, isOutput=False)
    bqp_d = nc.declare_dram_parameter("bqp", [128, 8], F32, isOutput=False)
    bkp_d = nc.declare_dram_parameter("bkp", [128, 8], F32, isOutput=False)
    bvr_d = nc.declare_dram_parameter("bvr", [1, D], F32, isOutput=False)
    bpr_d = nc.declare_dram_parameter("bpr", [1, D], F32, isOutput=False)
    out_d = nc.declare_dram_parameter("out", [QH, D], F32, isOutput=True)

    with ExitStack() as ctx:
        tc = ctx.enter_context(tile.TileContext(nc))

        const = ctx.enter_context(tc.tile_pool(name="const", bufs=1))
        ident = const.tile([128, 128], BF16)
        make_identity(nc, ident[:, :])
        ones1 = const.tile([1, 128], BF16)
        nc.vector.memset(ones1[:, :], 1.0)
        bqp = const.tile([128, 8], F32)
        nc.sync.dma_start(out=bqp[:, :], in_=bqp_d[:, :])
        bkp = const.tile([128, 8], F32)
        nc.sync.dma_start(out=bkp[:, :], in_=bkp_d[:, :])
        bvr = const.tile([1, D], BF16)
        nc.gpsimd.dma_start(out=bvr[:, :], in_=bvr_d[:, :])
        bpr = const.tile([1, D], BF16)
        nc.gpsimd.dma_start(out=bpr[:, :], in_=bpr_d[:, :])

        big = ctx.enter_context(tc.tile_pool(name="big", bufs=1))
        KT = big.tile([128, 8 * S], BF16)      # [p(2 heads), (j, k)]
        QT = big.tile([128, 8 * QH], BF16)     # [p(2 heads), (j, q)]
        Vaug = big.tile([128, 16 * 16 * 65], BF16)  # [p(s%128), (st, h, 65)]
        outT = big.tile([128, 8 * QH], BF16)   # [p(2 heads d), (j, q)]

        KTv = KT[:, :].rearrange("p (j k) -> p j k", j=8)
        QTv = QT[:, :].rearrange("p (j q) -> p j q", j=8)
        Vv = Vaug[:, :].rearrange("p (t h e) -> p t h e", t=16, h=16)
        oTv = outT[:, :].rearrange("p (j q) -> p j q", j=8)

        nc.gpsimd.memset(Vaug[:, :], 1.0)

        apool = ctx.enter_context(tc.tile_pool(name="att", bufs=4))
        npool = ctx.enter_context(tc.tile_pool(name="attn", bufs=1))
        xTp_cm = tc.tile_pool(name="xTp", bufs=1)
        xTp = xTp_cm.__enter__()
        xT = xTp.tile([128, 8 * S], BF16)      # [p, (dt, s)]
        xTv = xT[:, :].rearrange("p (d s) -> p d s", d=8)

        # Shared PSUM pools for the whole kernel
        psm = ctx.enter_context(tc.tile_pool(name="psm", bufs=3, space="PSUM"))
        pso = ctx.enter_context(tc.tile_pool(name="pso", bufs=2, space="PSUM"))

        # ---------------- Phase A: xT via PE transposes ----------------
        with tc.tile_pool(name="xstg", bufs=3) as xpool:
            for st in range(16):
                xb16 = xpool.tile([128, D], BF16, tag="xb16")
                nc.gpsimd.dma_start(out=xb16[:, :],
                                    in_=x_d[st * 128:(st + 1) * 128, :])
                pt = psm.tile([128, 1024], BF16, tag="ps", name=f"pt{st}")
                for dt_ in range(8):
                    nc.tensor.transpose(
                        pt[:, dt_ * 128:(dt_ + 1) * 128],
                        xb16[:, dt_ * 128:(dt_ + 1) * 128],
                        ident[:, :],
                    )
                dst = xTv[:, :, st * 128:(st + 1) * 128]
                src = pt[:, :].rearrange("p (d s) -> p d s", d=8)
                if st % 2 == 0:
                    nc.scalar.copy(dst, src)
                else:
                    nc.vector.tensor_copy(dst, src)

        def load_w(wd, pool, tag):
            # SWDGE casts f32 -> bf16 during the DMA
            tiles = []
            for dt_ in range(8):
                wb = pool.tile([128, D], BF16, tag=tag + "b" + str(dt_))
                nc.gpsimd.dma_start(out=wb[:, :],
                                    in_=wd[dt_ * 128:(dt_ + 1) * 128, :])
                tiles.append(wb)
            return tiles

        wkq_cm = tc.tile_pool(name="wkq", bufs=1)
        wkq = wkq_cm.__enter__()
        wkb = load_w(wk_d, wkq, "wk")
        wqb = load_w(wq_d, wkq, "wq")

        def kq_chunks(j):
            # 6 independent emit-steps (4 K s-chunks + 2 Q chunks), each
            # holding one PSUM slot for only ~8 matmuls
            steps = []
            for sc in range(4):
                def mk_k(sc=sc):
                    pkc = psm.tile([128, 512], F32, tag="ps",
                                   name=f"pk{j}_{sc}")
                    for dt_ in range(8):
                        nc.tensor.matmul(
                            pkc[:, :],
                            wkb[dt_][:, j * 128:(j + 1) * 128],
                            xTv[:, dt_, sc * 512:(sc + 1) * 512],
                            start=(dt_ == 0), stop=(dt_ == 7),
                        )
                    nc.vector.tensor_scalar_add(
                        KTv[:, j, sc * 512:(sc + 1) * 512], pkc[:, :],
                        bkp[:, j:j + 1])
                steps.append(mk_k)
            for qc in range(2):
                def mk_q(qc=qc):
                    pqc = psm.tile([128, 512], F32, tag="ps",
                                   name=f"pq{j}_{qc}")
                    for dt_ in range(8):
                        nc.tensor.matmul(
                            pqc[:, :],
                            wqb[dt_][:, j * 128:(j + 1) * 128],
                            xTv[:, dt_, qc * 512:(qc + 1) * 512],
                            start=(dt_ == 0), stop=(dt_ == 7),
                        )
                    nc.vector.tensor_scalar_add(
                        QTv[:, j, qc * 512:(qc + 1) * 512], pqc[:, :],
                        bqp[:, j:j + 1])
                steps.append(mk_q)
            return steps

        for step in kq_chunks(0):
            step()

        def attn_iter(j, qc, kt, poA, poB):
            qsl = slice(qc * 512, (qc + 1) * 512)
            ps = psm.tile([128, 1024], F32, tag="ps", name=f"ps{j}_{qc}_{kt}")
            nc.tensor.matmul(
                ps[:, 0:512],
                KTv[0:64, j, kt * 128:(kt + 1) * 128],
                QTv[0:64, j, qsl],
                start=True, stop=True, tile_position=(0, 0))
            nc.tensor.matmul(
                ps[:, 512:1024],
                KTv[64:128, j, kt * 128:(kt + 1) * 128],
                QTv[64:128, j, qsl],
                start=True, stop=True, tile_position=(64, 0))
            eP = apool.tile([128, 1024], BF16, tag="eP")
            nc.scalar.activation(eP[:, :], ps[:, :], Exp, scale=0.125)
            nc.tensor.matmul(
                poA[:, :], Vv[:, kt, 2 * j, 0:65], eP[:, 0:512],
                start=(kt == 0), stop=(kt == 15))
            nc.tensor.matmul(
                poB[:, :], Vv[:, kt, 2 * j + 1, 0:65], eP[:, 512:1024],
                start=(kt == 0), stop=(kt == 15))

        def attn_norm(j, qc, poA, poB):
            # fast unnormalized evacuation frees the PSUM accumulators;
            # normalization happens afterwards in SBUF on the DVE
            qsl = slice(qc * 512, (qc + 1) * 512)
            lp = npool.tile([1, 1024], F32, tag="lp")
            nc.vector.tensor_copy(lp[0:1, 0:512], poA[64:65, :])
            nc.vector.tensor_copy(lp[0:1, 512:1024], poB[64:65, :])
            nc.vector.tensor_copy(oTv[0:64, j, qsl], poA[0:64, :])
            nc.vector.tensor_copy(oTv[64:128, j, qsl], poB[0:64, :])
            rp = npool.tile([1, 1024], F32, tag="rp")
            nc.vector.reciprocal_approx_fast(rp[:, :], lp[:, :])
            rpb = npool.tile([1, 1024], BF16, tag="rpb")
            nc.vector.tensor_copy(rpb[:, :], rp[:, :])
            pbc = psm.tile([128, 1024], F32, tag="ps", name=f"pbc{j}_{qc}")
            nc.tensor.matmul(pbc[0:64, 0:512], ones1[0:1, 0:64],
                             rpb[0:1, 0:512], start=True, stop=True)
            nc.tensor.matmul(pbc[64:128, 0:512], ones1[0:1, 0:64],
                             rpb[0:1, 512:1024], start=True, stop=True,
                             tile_position=(0, 64))
            rbc = npool.tile([128, 512], F32, tag="rbc")
            nc.vector.tensor_copy(rbc[:, :], pbc[:, 0:512])
            nc.vector.tensor_tensor(
                oTv[0:64, j, qsl], oTv[0:64, j, qsl], rbc[0:64, :], MULT)
            nc.vector.tensor_tensor(
                oTv[64:128, j, qsl], oTv[64:128, j, qsl], rbc[64:128, :],
                MULT)

        def attn_block(j, qc, interleave=None):
            poA = pso.tile([65, 512], F32, tag="po", name=f"poA{j}_{qc}")
            poB = pso.tile([65, 512], F32, tag="po", name=f"poB{j}_{qc}")
            nsteps = len(interleave) if interleave else 0
            si = 0
            for kt in range(16):
                if interleave and kt % 3 == 0 and si < nsteps:
                    interleave[si]()
                    si += 1
                attn_iter(j, qc, kt, poA, poB)
            while interleave and si < nsteps:
                interleave[si]()
                si += 1
            attn_norm(j, qc, poA, poB)

        # V proj pipelined with the first attention block (attnV(kt) only
        # needs Vaug[st=kt], which V(st) just produced)
        with tc.tile_pool(name="wv", bufs=1) as wvp:
            wvb = load_w(wv_d, wvp, "wv")

            def v_st(st):
                pv = psm.tile([128, 1024], F32, tag="ps", name=f"pv{st}")
                for nh in range(2):
                    nc.tensor.matmul(
                        pv[:, nh * 512:(nh + 1) * 512], ones1[:, :],
                        bvr[:, nh * 512:(nh + 1) * 512], start=True,
                        stop=False)
                for dt_ in range(8):
                    for nh in range(2):
                        nc.tensor.matmul(
                            pv[:, nh * 512:(nh + 1) * 512],
                            xTv[:, dt_, st * 128:(st + 1) * 128],
                            wvb[dt_][:, nh * 512:(nh + 1) * 512],
                            start=False, stop=(dt_ == 7),
                        )
                dst = Vv[:, st, :, 0:64]
                src_ = pv[:, :].rearrange("p (h d) -> p h d", h=16)
                if st % 2 == 0:
                    nc.scalar.copy(dst, src_)
                else:
                    nc.vector.tensor_copy(dst, src_)

            poA0 = pso.tile([65, 512], F32, tag="po", name="poA0_0")
            poB0 = pso.tile([65, 512], F32, tag="po", name="poB0_0")
            for st in range(16):
                v_st(st)
                attn_iter(0, 0, st, poA0, poB0)
            attn_norm(0, 0, poA0, poB0)

        for step in kq_chunks(1):
            step()
        for j in range(1, 8):
            attn_block(j, 0,
                       interleave=kq_chunks(j + 1) if j < 7 else None)
        wkq_cm.__exit__(None, None, None)
        xTp_cm.__exit__(None, None, None)

        # ---------------- Phase D: proj interleaved with qc=1 attention ---
        with tc.tile_pool(name="wp", bufs=1) as wpp, \
             tc.tile_pool(name="ystg", bufs=2) as ypool:
            wpb = load_w(wp_d, wpp, "wp")

            def proj(qt):
                py = psm.tile([128, 1024], F32, tag="ps", name=f"py{qt}")
                for nh in range(2):
                    nc.tensor.matmul(py[:, nh * 512:(nh + 1) * 512],
                                     ones1[:, :],
                                     bpr[:, nh * 512:(nh + 1) * 512],
                                     start=True, stop=False)
                for j in range(8):
                    for nh in range(2):
                        nc.tensor.matmul(
                            py[:, nh * 512:(nh + 1) * 512],
                            oTv[:, j, qt * 128:(qt + 1) * 128],
                            wpb[j][:, nh * 512:(nh + 1) * 512],
                            start=False, stop=(j == 7),
                        )
                ys = ypool.tile([128, 1024], F32, tag="ys")
                nc.scalar.copy(ys[:, :], py[:, :])
                nc.sync.dma_start(
                    out=out_d[qt * 128:(qt + 1) * 128, :], in_=ys[:, :])

            for j in range(8):
                attn_block(j, 1)
                if j >= 4:
                    proj(j - 4)
            for qt in range(4, 8):
                proj(qt)

    nc.finalize()
    return nc


def _in_maps(x, W_qkv, b_qkv, W_proj, b_proj):
    x = np.asarray(x, np.float32)
    W_qkv = np.asarray(W_qkv, np.float32)
    b_qkv = np.asarray(b_qkv, np.float32)
    W_proj = np.ascontiguousarray(np.asarray(W_proj, np.float32))
    b_proj = np.asarray(b_proj, np.float32)
    Wq = np.ascontiguousarray(W_qkv[:, 0:D])
    Wk = np.ascontiguousarray(W_qkv[:, D:2 * D])
    Wv = np.ascontiguousarray(W_qkv[:, 2 * D:3 * D])
    bq, bk, bv = b_qkv[0:D], b_qkv[D:2 * D], b_qkv[2 * D:3 * D]
    bqp = np.ascontiguousarray(bq.reshape(8, 128).T)
    bkp = np.ascontiguousarray(bk.reshape(8, 128).T)
    maps = []
    for c in range(NC_):
        b, qh = c // 2, c % 2
        xb = np.concatenate(
            [x[b, qh * QH:(qh + 1) * QH], x[b, (1 - qh) * QH:(2 - qh) * QH]],
            axis=0)
        maps.append({
            "xb": np.ascontiguousarray(xb), "wq": Wq, "wk": Wk, "wv": Wv,
            "wp": W_proj, "bqp": bqp, "bkp": bkp,
            "bvr": np.ascontiguousarray(bv.reshape(1, D)),
            "bpr": np.ascontiguousarray(b_proj.reshape(1, D)),
        })
    return maps


def run(x, W_qkv, b_qkv, W_proj, b_proj, trace=False, tmpdir=None):
    sys.path.insert(0, "/opt/trn_rl_repo")
    from concourse.bass_utils import run_bass_kernel_spmd

    if "nc" not in _cache:
        _cache["nc"] = _build_nc()
    nc = _cache["nc"]
    maps = _in_maps(x, W_qkv, b_qkv, W_proj, b_proj)
    res = run_bass_kernel_spmd(nc, maps, core_ids=list(range(NC_)),
                               trace=trace, tmpdir=tmpdir)
    y = np.empty((B, S, D), np.float32)
    for c in range(NC_):
        b, qh = c // 2, c % 2
        y[b, qh * QH:(qh + 1) * QH] = res.results[c]["out"]
    return y, res


def kernel(x, W_qkv, b_qkv, W_proj, b_proj):
    y, _ = run(x, W_qkv, b_qkv, W_proj, b_proj, trace=False)
    return y


# revision 24
# speedup vs baseline: 1.1376x; 1.1376x over previous
"""Trainium2 Bass kernel for nn_Attention (B=4, S=2048, D=1024, H=16, hd=64, fp32).

Sharding (zero-communication): 8 cores; core c handles batch b=c//2 and
query-half qh=c%2. Each core computes K,V for its whole batch (all heads),
Q for its query half, attention for all 16 heads over its 1024 queries, and
the output projection for its 1024 rows. The per-core input x is permuted so
the core's query half comes first (softmax over keys is permutation
invariant, so K/V may use the permuted order as long as they agree).

Per-core pipeline (all matmuls bf16, accumulation fp32 in PSUM):
  A. xT[D,S] built via PE transposes of x tiles (bf16).
  B. KT[hd,S] / QT[hd,Sq] (heads stacked 2-per-128-partitions), V[S,hd]
     augmented with a ones column (gives the softmax denominator for free).
     Emission order K0,Q0 -> V -> K1..7,Q1..7 interleaved with attention so
     ScalarE exp work starts as early as possible.
  C. scoresT[k,q] via PE (two heads row-packed with tile_position), exp on
     ScalarE (no max subtraction: |scores/8| < ~3 by construction), attnV
     accumulates (P @ V)^T; the ones column produces l[q]; normalization via
     reciprocal + K=1 broadcast matmuls fused into the PSUM evacuation.
  D. y = outT^T @ W_proj + b_proj (bias via K=1 ones matmul).

One shared [128,1024] PSUM pool (3 slots) + a [65,512] accumulator pool
(2 slots) keeps all phases inside the 8 PSUM banks with fine-grained
slot-level WAR deps instead of phase barriers.
"""

import os
import sys

import numpy as np

B, S, D, H, HD = 4, 2048, 1024, 16, 64
QH = 1024  # queries per core
NC_ = 8

_cache = {}


def _build_nc():
    sys.path.insert(0, "/opt/trn_rl_repo")
    import concourse.bass as bass
    from concourse import bacc
    import concourse.mybir as mybir
    import concourse.tile as tile
    from concourse.masks import make_identity
    from contextlib import ExitStack

    F32 = mybir.dt.float32
    BF16 = mybir.dt.bfloat16
    MULT = mybir.AluOpType.mult
    Exp = mybir.ActivationFunctionType.Exp

    nc = bacc.Bacc()
    x_d = nc.declare_dram_parameter("xb", [S, D], F32, isOutput=False)
    wq_d = nc.declare_dram_parameter("wq", [D, D], F32, isOutput=False)
    wk_d = nc.declare_dram_parameter("wk", [D, D], F32, isOutput=False)
    wv_d = nc.declare_dram_parameter("wv", [D, D], F32, isOutput=False)
    wp_d = nc.declare_dram_parameter("wp", [D, D], F32, isOutput=False)
    bqp_d = nc.declare_dram_parameter("bqp", [128, 8], F32, isOutput=False)
    bkp_d = nc.declare_dram_parameter("bkp", [128, 8], F32, isOutput=False)
    bvr_d = nc.declare_dram_parameter("bvr", [1, D], F32, isOutput=False)
    bpr_d = nc.declare_dram_parameter("bpr", [1, D], F32, isOutput=False)
    out_d = nc.declare_dram_parameter("out", [QH, D], F32, isOutput=True)

    with ExitStack() as ctx:
        tc = ctx.enter_context(tile.TileContext(nc))

        const = ctx.enter_context(tc.tile_pool(name="const", bufs=1))
        ident = const.tile([128, 128], BF16)
        make_identity(nc, ident[:, :])
        ones1 = const.tile([1, 128], BF16)
        nc.vector.memset(ones1[:, :], 1.0)
        bqp = const.tile([128, 8], F32)
        nc.sync.dma_start(out=bqp[:, :], in_=bqp_d[:, :])
        bkp = const.tile([128, 8], F32)
        nc.sync.dma_start(out=bkp[:, :], in_=bkp_d[:, :])
        bvr = const.tile([1, D], BF16)
        nc.gpsimd.dma_start(out=bvr[:, :], in_=bvr_d[:, :])
        bpr = const.tile([1, D], BF16)
        nc.gpsimd.dma_start(out=bpr[:, :], in_=bpr_d[:, :])

        big = ctx.enter_context(tc.tile_pool(name="big", bufs=1))
        KT = big.tile([128, 8 * S], BF16)      # [p(2 heads), (j, k)]
        QT = big.tile([128, 8 * QH], BF16)     # [p(2 heads), (j, q)]
        Vaug = big.tile([128, 16 * 16 * 65], BF16)  # [p(s%128), (st, h, 65)]
        outT = big.tile([128, 8 * QH], BF16)   # [p(2 heads d), (j, q)]

        KTv = KT[:, :].rearrange("p (j k) -> p j k", j=8)
        QTv = QT[:, :].rearrange("p (j q) -> p j q", j=8)
        Vv = Vaug[:, :].rearrange("p (t h e) -> p t h e", t=16, h=16)
        oTv = outT[:, :].rearrange("p (j q) -> p j q", j=8)

        nc.gpsimd.memset(Vaug[:, :], 1.0)

        apool = ctx.enter_context(tc.tile_pool(name="att", bufs=4))
        npool = ctx.enter_context(tc.tile_pool(name="attn", bufs=1))
        xTp_cm = tc.tile_pool(name="xTp", bufs=1)
        xTp = xTp_cm.__enter__()
        xT = xTp.tile([128, 8 * S], BF16)      # [p, (dt, s)]
        xTv = xT[:, :].rearrange("p (d s) -> p d s", d=8)

        # Shared PSUM pools for the whole kernel
        psm = ctx.enter_context(tc.tile_pool(name="psm", bufs=3, space="PSUM"))
        pso = ctx.enter_context(tc.tile_pool(name="pso", bufs=2, space="PSUM"))

        # ---------------- Phase A: xT via PE transposes ----------------
        with tc.tile_pool(name="xstg", bufs=3) as xpool:
            for st in range(16):
                xb16 = xpool.tile([128, D], BF16, tag="xb16")
                nc.gpsimd.dma_start(out=xb16[:, :],
                                    in_=x_d[st * 128:(st + 1) * 128, :])
                pt = psm.tile([128, 1024], BF16, tag="ps", name=f"pt{st}")
                for dt_ in range(8):
                    nc.tensor.transpose(
                        pt[:, dt_ * 128:(dt_ + 1) * 128],
                        xb16[:, dt_ * 128:(dt_ + 1) * 128],
                        ident[:, :],
                    )
                dst = xTv[:, :, st * 128:(st + 1) * 128]
                src = pt[:, :].rearrange("p (d s) -> p d s", d=8)
                if st % 2 == 0:
                    nc.scalar.copy(dst, src)
                else:
                    nc.vector.tensor_copy(dst, src)

        def load_w(wd, pool, tag):
            # SWDGE casts f32 -> bf16 during the DMA
            tiles = []
            for dt_ in range(8):
                wb = pool.tile([128, D], BF16, tag=tag + "b" + str(dt_))
                nc.gpsimd.dma_start(out=wb[:, :],
                                    in_=wd[dt_ * 128:(dt_ + 1) * 128, :])
                tiles.append(wb)
            return tiles

        wkq_cm = tc.tile_pool(name="wkq", bufs=1)
        wkq = wkq_cm.__enter__()
        wkb = load_w(wk_d, wkq, "wk")
        wqb = load_w(wq_d, wkq, "wq")

        def kq_chunks(j):
            # 6 independent emit-steps (4 K s-chunks + 2 Q chunks), each
            # holding one PSUM slot for only ~8 matmuls
            steps = []
            for sc in range(4):
                def mk_k(sc=sc):
                    pkc = psm.tile([128, 512], F32, tag="ps",
                                   name=f"pk{j}_{sc}")
                    for dt_ in range(8):
                        nc.tensor.matmul(
                            pkc[:, :],
                            wkb[dt_][:, j * 128:(j + 1) * 128],
                            xTv[:, dt_, sc * 512:(sc + 1) * 512],
                            start=(dt_ == 0), stop=(dt_ == 7),
                        )
                    nc.vector.tensor_scalar_add(
                        KTv[:, j, sc * 512:(sc + 1) * 512], pkc[:, :],
                        bkp[:, j:j + 1])
                steps.append(mk_k)
            for qc in range(2):
                def mk_q(qc=qc):
                    pqc = psm.tile([128, 512], F32, tag="ps",
                                   name=f"pq{j}_{qc}")
                    for dt_ in range(8):
                        nc.tensor.matmul(
                            pqc[:, :],
                            wqb[dt_][:, j * 128:(j + 1) * 128],
                            xTv[:, dt_, qc * 512:(qc + 1) * 512],
                            start=(dt_ == 0), stop=(dt_ == 7),
                        )
                    nc.vector.tensor_scalar_add(
                        QTv[:, j, qc * 512:(qc + 1) * 512], pqc[:, :],
                        bqp[:, j:j + 1])
                steps.append(mk_q)
            return steps

        for step in kq_chunks(0):
            step()

        def attn_iter(j, qc, kt, poA, poB):
            qsl = slice(qc * 512, (qc + 1) * 512)
            ps = psm.tile([128, 1024], F32, tag="ps", name=f"ps{j}_{qc}_{kt}")
            nc.tensor.matmul(
                ps[:, 0:512],
                KTv[0:64, j, kt * 128:(kt + 1) * 128],
                QTv[0:64, j, qsl],
                start=True, stop=True, tile_position=(0, 0))
            nc.tensor.matmul(
                ps[:, 512:1024],
                KTv[64:128, j, kt * 128:(kt + 1) * 128],
                QTv[64:128, j, qsl],
                start=True, stop=True, tile_position=(64, 0))
            eP = apool.tile([128, 1024], BF16, tag="eP")
            nc.scalar.activation(eP[:, :], ps[:, :], Exp, scale=0.125)
            nc.tensor.matmul(
                poA[:, :], Vv[:, kt, 2 * j, 0:65], eP[:, 0:512],
                start=(kt == 0), stop=(kt == 15))
            nc.tensor.matmul(
                poB[:, :], Vv[:, kt, 2 * j + 1, 0:65], eP[:, 512:1024],
                start=(kt == 0), stop=(kt == 15))

        rpbs = {}

        def attn_fast_evac(j, qc, poA, poB):
            # unnormalized evacuation frees the PSUM accumulators fast;
            # the reciprocal chain runs on the DVE off the critical path
            qsl = slice(qc * 512, (qc + 1) * 512)
            lp = npool.tile([1, 1024], F32, tag="lp", name=f"lp{j}_{qc}")
            nc.vector.tensor_copy(lp[0:1, 0:512], poA[64:65, :])
            nc.vector.tensor_copy(lp[0:1, 512:1024], poB[64:65, :])
            nc.vector.tensor_copy(oTv[0:64, j, qsl], poA[0:64, :])
            nc.vector.tensor_copy(oTv[64:128, j, qsl], poB[0:64, :])
            rp = npool.tile([1, 1024], F32, tag="rp", name=f"rp{j}_{qc}")
            nc.vector.reciprocal_approx_fast(rp[:, :], lp[:, :])
            rpb = npool.tile([1, 1024], BF16, tag="rpb", name=f"rpb{j}_{qc}")
            nc.vector.tensor_copy(rpb[:, :], rp[:, :])
            rpbs[(j, qc)] = rpb

        def attn_norm_tail(j, qc):
            qsl = slice(qc * 512, (qc + 1) * 512)
            rpb = rpbs.pop((j, qc))
            pbc = psm.tile([128, 1024], F32, tag="ps", name=f"pbc{j}_{qc}")
            nc.tensor.matmul(pbc[0:64, 0:512], ones1[0:1, 0:64],
                             rpb[0:1, 0:512], start=True, stop=True)
            nc.tensor.matmul(pbc[64:128, 0:512], ones1[0:1, 0:64],
                             rpb[0:1, 512:1024], start=True, stop=True,
                             tile_position=(0, 64))
            rbc = npool.tile([128, 512], F32, tag="rbc", name=f"rbc{j}_{qc}")
            nc.vector.tensor_copy(rbc[:, :], pbc[:, 0:512])
            nc.vector.tensor_tensor(
                oTv[0:64, j, qsl], oTv[0:64, j, qsl], rbc[0:64, :], MULT)
            nc.vector.tensor_tensor(
                oTv[64:128, j, qsl], oTv[64:128, j, qsl], rbc[64:128, :],
                MULT)

        pending_norm = []

        def flush_norm():
            while pending_norm:
                pending_norm.pop(0)()

        def attn_block(j, qc, interleave=None):
            poA = pso.tile([65, 512], F32, tag="po", name=f"poA{j}_{qc}")
            poB = pso.tile([65, 512], F32, tag="po", name=f"poB{j}_{qc}")
            nsteps = len(interleave) if interleave else 0
            si = 0
            for kt in range(16):
                if interleave and kt % 3 == 0 and si < nsteps:
                    interleave[si]()
                    si += 1
                attn_iter(j, qc, kt, poA, poB)
                if kt == 2:
                    # run the previous block's deferred normalization now:
                    # its DVE chain is long done, so the PE-side broadcast
                    # matmul no longer stalls the engine stream
                    flush_norm()
            while interleave and si < nsteps:
                interleave[si]()
                si += 1
            attn_fast_evac(j, qc, poA, poB)
            pending_norm.append(lambda j=j, qc=qc: attn_norm_tail(j, qc))

        # V proj pipelined with the first attention block (attnV(kt) only
        # needs Vaug[st=kt], which V(st) just produced)
        with tc.tile_pool(name="wv", bufs=1) as wvp:
            wvb = load_w(wv_d, wvp, "wv")

            def v_st(st):
                pv = psm.tile([128, 1024], F32, tag="ps", name=f"pv{st}")
                for nh in range(2):
                    nc.tensor.matmul(
                        pv[:, nh * 512:(nh + 1) * 512], ones1[:, :],
                        bvr[:, nh * 512:(nh + 1) * 512], start=True,
                        stop=False)
                for dt_ in range(8):
                    for nh in range(2):
                        nc.tensor.matmul(
                            pv[:, nh * 512:(nh + 1) * 512],
                            xTv[:, dt_, st * 128:(st + 1) * 128],
                            wvb[dt_][:, nh * 512:(nh + 1) * 512],
                            start=False, stop=(dt_ == 7),
                        )
                dst = Vv[:, st, :, 0:64]
                src_ = pv[:, :].rearrange("p (h d) -> p h d", h=16)
                if st % 2 == 0:
                    nc.scalar.copy(dst, src_)
                else:
                    nc.vector.tensor_copy(dst, src_)

            poA0 = pso.tile([65, 512], F32, tag="po", name="poA0_0")
            poB0 = pso.tile([65, 512], F32, tag="po", name="poB0_0")
            for st in range(16):
                v_st(st)
                attn_iter(0, 0, st, poA0, poB0)
            attn_fast_evac(0, 0, poA0, poB0)
            pending_norm.append(lambda: attn_norm_tail(0, 0))

        for step in kq_chunks(1):
            step()
        for j in range(1, 8):
            attn_block(j, 0,
                       interleave=kq_chunks(j + 1) if j < 7 else None)
        flush_norm()
        wkq_cm.__exit__(None, None, None)
        xTp_cm.__exit__(None, None, None)

        # ---------------- Phase D: proj interleaved with qc=1 attention ---
        with tc.tile_pool(name="wp", bufs=1) as wpp, \
             tc.tile_pool(name="ystg", bufs=2) as ypool:
            wpb = load_w(wp_d, wpp, "wp")

            def proj(qt):
                py = psm.tile([128, 1024], F32, tag="ps", name=f"py{qt}")
                for nh in range(2):
                    nc.tensor.matmul(py[:, nh * 512:(nh + 1) * 512],
                                     ones1[:, :],
                                     bpr[:, nh * 512:(nh + 1) * 512],
                                     start=True, stop=False)
                for j in range(8):
                    for nh in range(2):
                        nc.tensor.matmul(
                            py[:, nh * 512:(nh + 1) * 512],
                            oTv[:, j, qt * 128:(qt + 1) * 128],
                            wpb[j][:, nh * 512:(nh + 1) * 512],
                            start=False, stop=(j == 7),
                        )
                ys = ypool.tile([128, 1024], F32, tag="ys")
                nc.scalar.copy(ys[:, :], py[:, :])
                nc.sync.dma_start(
                    out=out_d[qt * 128:(qt + 1) * 128, :], in_=ys[:, :])

            for j in range(8):
                attn_block(j, 1)
                if j >= 4:
                    proj(j - 4)
            flush_norm()
            for qt in range(4, 8):
                proj(qt)

    nc.finalize()
    return nc


def _in_maps(x, W_qkv, b_qkv, W_proj, b_proj):
    x = np.asarray(x, np.float32)
    W_qkv = np.asarray(W_qkv, np.float32)
    b_qkv = np.asarray(b_qkv, np.float32)
    W_proj = np.ascontiguousarray(np.asarray(W_proj, np.float32))
    b_proj = np.asarray(b_proj, np.float32)
    Wq = np.ascontiguousarray(W_qkv[:, 0:D])
    Wk = np.ascontiguousarray(W_qkv[:, D:2 * D])
    Wv = np.ascontiguousarray(W_qkv[:, 2 * D:3 * D])
    bq, bk, bv = b_qkv[0:D], b_qkv[D:2 * D], b_qkv[2 * D:3 * D]
    bqp = np.ascontiguousarray(bq.reshape(8, 128).T)
    bkp = np.ascontiguousarray(bk.reshape(8, 128).T)
    maps = []
    for c in range(NC_):
        b, qh = c // 2, c % 2
        xb = np.concatenate(
            [x[b, qh * QH:(qh + 1) * QH], x[b, (1 - qh) * QH:(2 - qh) * QH]],
            axis=0)
        maps.append({
            "xb": np.ascontiguousarray(xb), "wq": Wq, "wk": Wk, "wv": Wv,
            "wp": W_proj, "bqp": bqp, "bkp": bkp,
            "bvr": np.ascontiguousarray(bv.reshape(1, D)),
            "bpr": np.ascontiguousarray(b_proj.reshape(1, D)),
        })
    return maps


def run(x, W_qkv, b_qkv, W_proj, b_proj, trace=False, tmpdir=None):
    sys.path.insert(0, "/opt/trn_rl_repo")
    from concourse.bass_utils import run_bass_kernel_spmd

    if "nc" not in _cache:
        _cache["nc"] = _build_nc()
    nc = _cache["nc"]
    maps = _in_maps(x, W_qkv, b_qkv, W_proj, b_proj)
    res = run_bass_kernel_spmd(nc, maps, core_ids=list(range(NC_)),
                               trace=trace, tmpdir=tmpdir)
    y = np.empty((B, S, D), np.float32)
    for c in range(NC_):
        b, qh = c // 2, c % 2
        y[b, qh * QH:(qh + 1) * QH] = res.results[c]["out"]
    return y, res


def kernel(x, W_qkv, b_qkv, W_proj, b_proj):
    y, _ = run(x, W_qkv, b_qkv, W_proj, b_proj, trace=False)
    return y


# revision 25
# speedup vs baseline: 1.1585x; 1.0184x over previous
"""Trainium2 Bass kernel for nn_Attention (B=4, S=2048, D=1024, H=16, hd=64, fp32).

Sharding (zero-communication): 8 cores; core c handles batch b=c//2 and
query-half qh=c%2. Each core computes K,V for its whole batch (all heads),
Q for its query half, attention for all 16 heads over its 1024 queries, and
the output projection for its 1024 rows. The per-core input x is permuted so
the core's query half comes first (softmax over keys is permutation
invariant, so K/V may use the permuted order as long as they agree).

Per-core pipeline (all matmuls bf16, accumulation fp32 in PSUM):
  A. xT[D,S] built via PE transposes of x tiles (bf16).
  B. KT[hd,S] / QT[hd,Sq] (heads stacked 2-per-128-partitions), V[S,hd]
     augmented with a ones column (gives the softmax denominator for free).
     Emission order K0,Q0 -> V -> K1..7,Q1..7 interleaved with attention so
     ScalarE exp work starts as early as possible.
  C. scoresT[k,q] via PE (two heads row-packed with tile_position), exp on
     ScalarE (no max subtraction: |scores/8| < ~3 by construction), attnV
     accumulates (P @ V)^T; the ones column produces l[q]; normalization via
     reciprocal + K=1 broadcast matmuls fused into the PSUM evacuation.
  D. y = outT^T @ W_proj + b_proj (bias via K=1 ones matmul).

One shared [128,1024] PSUM pool (3 slots) + a [65,512] accumulator pool
(2 slots) keeps all phases inside the 8 PSUM banks with fine-grained
slot-level WAR deps instead of phase barriers.
"""

import os
import sys

import numpy as np

B, S, D, H, HD = 4, 2048, 1024, 16, 64
QH = 1024  # queries per core
NC_ = 8

_cache = {}


def _build_nc():
    sys.path.insert(0, "/opt/trn_rl_repo")
    import concourse.bass as bass
    from concourse import bacc
    import concourse.mybir as mybir
    import concourse.tile as tile
    from concourse.masks import make_identity
    from contextlib import ExitStack

    F32 = mybir.dt.float32
    BF16 = mybir.dt.bfloat16
    MULT = mybir.AluOpType.mult
    Exp = mybir.ActivationFunctionType.Exp

    nc = bacc.Bacc()
    x_d = nc.declare_dram_parameter("xb", [S, D], F32, isOutput=False)
    wq_d = nc.declare_dram_parameter("wq", [D, D], F32, isOutput=False)
    wk_d = nc.declare_dram_parameter("wk", [D, D], F32, isOutput=False)
    wv_d = nc.declare_dram_parameter("wv", [D, D], F32, isOutput=False)
    wp_d = nc.declare_dram_parameter("wp", [D, D], F32, isOutput=False)
    bqp_d = nc.declare_dram_parameter("bqp", [128, 8], F32, isOutput=False)
    bkp_d = nc.declare_dram_parameter("bkp", [128, 8], F32, isOutput=False)
    bvr_d = nc.declare_dram_parameter("bvr", [1, D], F32, isOutput=False)
    bpr_d = nc.declare_dram_parameter("bpr", [1, D], F32, isOutput=False)
    out_d = nc.declare_dram_parameter("out", [QH, D], F32, isOutput=True)

    with ExitStack() as ctx:
        tc = ctx.enter_context(tile.TileContext(nc))

        const = ctx.enter_context(tc.tile_pool(name="const", bufs=1))
        ident = const.tile([128, 128], BF16)
        make_identity(nc, ident[:, :])
        ones1 = const.tile([1, 128], BF16)
        nc.vector.memset(ones1[:, :], 1.0)
        bqp = const.tile([128, 8], F32)
        nc.sync.dma_start(out=bqp[:, :], in_=bqp_d[:, :])
        bkp = const.tile([128, 8], F32)
        nc.sync.dma_start(out=bkp[:, :], in_=bkp_d[:, :])
        bvr = const.tile([1, D], BF16)
        nc.gpsimd.dma_start(out=bvr[:, :], in_=bvr_d[:, :])
        bpr = const.tile([1, D], BF16)
        nc.gpsimd.dma_start(out=bpr[:, :], in_=bpr_d[:, :])

        big = ctx.enter_context(tc.tile_pool(name="big", bufs=1))
        KT = big.tile([128, 8 * S], BF16)      # [p(2 heads), (j, k)]
        QT = big.tile([128, 8 * QH], BF16)     # [p(2 heads), (j, q)]
        Vaug = big.tile([128, 16 * 16 * 65], BF16)  # [p(s%128), (st, h, 65)]
        outT = big.tile([128, 8 * QH], BF16)   # [p(2 heads d), (j, q)]

        KTv = KT[:, :].rearrange("p (j k) -> p j k", j=8)
        QTv = QT[:, :].rearrange("p (j q) -> p j q", j=8)
        Vv = Vaug[:, :].rearrange("p (t h e) -> p t h e", t=16, h=16)
        oTv = outT[:, :].rearrange("p (j q) -> p j q", j=8)

        nc.gpsimd.memset(Vaug[:, :], 1.0)

        apool = ctx.enter_context(tc.tile_pool(name="att", bufs=4))
        npool = ctx.enter_context(tc.tile_pool(name="attn", bufs=1))
        xTp_cm = tc.tile_pool(name="xTp", bufs=1)
        xTp = xTp_cm.__enter__()
        xT = xTp.tile([128, 8 * S], BF16)      # [p, (dt, s)]
        xTv = xT[:, :].rearrange("p (d s) -> p d s", d=8)

        # Shared PSUM pools for the whole kernel
        psm = ctx.enter_context(tc.tile_pool(name="psm", bufs=3, space="PSUM"))
        pso = ctx.enter_context(tc.tile_pool(name="pso", bufs=2, space="PSUM"))

        # ---------------- Phase A: xT via PE transposes ----------------
        with tc.tile_pool(name="xstg", bufs=5) as xpool:
            for st in range(16):
                xb16 = xpool.tile([128, D], BF16, tag="xb16")
                nc.gpsimd.dma_start(out=xb16[:, :],
                                    in_=x_d[st * 128:(st + 1) * 128, :])
                pt = psm.tile([128, 1024], BF16, tag="ps", name=f"pt{st}")
                for dt_ in range(8):
                    nc.tensor.transpose(
                        pt[:, dt_ * 128:(dt_ + 1) * 128],
                        xb16[:, dt_ * 128:(dt_ + 1) * 128],
                        ident[:, :],
                    )
                dst = xTv[:, :, st * 128:(st + 1) * 128]
                src = pt[:, :].rearrange("p (d s) -> p d s", d=8)
                if st % 2 == 0:
                    nc.scalar.copy(dst, src)
                else:
                    nc.vector.tensor_copy(dst, src)

        def load_w(wd, pool, tag):
            # SWDGE casts f32 -> bf16 during the DMA
            tiles = []
            for dt_ in range(8):
                wb = pool.tile([128, D], BF16, tag=tag + "b" + str(dt_))
                nc.gpsimd.dma_start(out=wb[:, :],
                                    in_=wd[dt_ * 128:(dt_ + 1) * 128, :])
                tiles.append(wb)
            return tiles

        wkq_cm = tc.tile_pool(name="wkq", bufs=1)
        wkq = wkq_cm.__enter__()
        wkb = load_w(wk_d, wkq, "wk")
        wqb = load_w(wq_d, wkq, "wq")

        def kq_chunks(j):
            # 6 independent emit-steps (4 K s-chunks + 2 Q chunks), each
            # holding one PSUM slot for only ~8 matmuls
            steps = []
            ksteps = []
            for sc in range(4):
                def mk_k(sc=sc):
                    pkc = psm.tile([128, 512], F32, tag="ps",
                                   name=f"pk{j}_{sc}")
                    for dt_ in range(8):
                        nc.tensor.matmul(
                            pkc[:, :],
                            wkb[dt_][:, j * 128:(j + 1) * 128],
                            xTv[:, dt_, sc * 512:(sc + 1) * 512],
                            start=(dt_ == 0), stop=(dt_ == 7),
                        )
                    nc.vector.tensor_scalar_add(
                        KTv[:, j, sc * 512:(sc + 1) * 512], pkc[:, :],
                        bkp[:, j:j + 1])
                ksteps.append(mk_k)
            for qc in range(2):
                def mk_q(qc=qc):
                    pqc = psm.tile([128, 512], F32, tag="ps",
                                   name=f"pq{j}_{qc}")
                    for dt_ in range(8):
                        nc.tensor.matmul(
                            pqc[:, :],
                            wqb[dt_][:, j * 128:(j + 1) * 128],
                            xTv[:, dt_, qc * 512:(qc + 1) * 512],
                            start=(dt_ == 0), stop=(dt_ == 7),
                        )
                    nc.vector.tensor_scalar_add(
                        QTv[:, j, qc * 512:(qc + 1) * 512], pqc[:, :],
                        bqp[:, j:j + 1])
                steps.append(mk_q)
            # K0 then both Q chunks first: unblocks the next block's scores
            # (and the very first exp) as early as possible
            return [ksteps[0]] + steps + ksteps[1:]

        for step in kq_chunks(0):
            step()

        def attn_iter(j, qc, kt, poA, poB):
            qsl = slice(qc * 512, (qc + 1) * 512)
            ps = psm.tile([128, 1024], F32, tag="ps", name=f"ps{j}_{qc}_{kt}")
            nc.tensor.matmul(
                ps[:, 0:512],
                KTv[0:64, j, kt * 128:(kt + 1) * 128],
                QTv[0:64, j, qsl],
                start=True, stop=True, tile_position=(0, 0))
            nc.tensor.matmul(
                ps[:, 512:1024],
                KTv[64:128, j, kt * 128:(kt + 1) * 128],
                QTv[64:128, j, qsl],
                start=True, stop=True, tile_position=(64, 0))
            eP = apool.tile([128, 1024], BF16, tag="eP")
            nc.scalar.activation(eP[:, :], ps[:, :], Exp, scale=0.125)
            nc.tensor.matmul(
                poA[:, :], Vv[:, kt, 2 * j, 0:65], eP[:, 0:512],
                start=(kt == 0), stop=(kt == 15))
            nc.tensor.matmul(
                poB[:, :], Vv[:, kt, 2 * j + 1, 0:65], eP[:, 512:1024],
                start=(kt == 0), stop=(kt == 15))

        rpbs = {}

        def attn_fast_evac(j, qc, poA, poB):
            # unnormalized evacuation frees the PSUM accumulators fast;
            # the reciprocal chain runs on the DVE off the critical path
            qsl = slice(qc * 512, (qc + 1) * 512)
            lp = npool.tile([1, 1024], F32, tag="lp", name=f"lp{j}_{qc}")
            nc.vector.tensor_copy(lp[0:1, 0:512], poA[64:65, :])
            nc.vector.tensor_copy(lp[0:1, 512:1024], poB[64:65, :])
            nc.vector.tensor_copy(oTv[0:64, j, qsl], poA[0:64, :])
            nc.vector.tensor_copy(oTv[64:128, j, qsl], poB[0:64, :])
            rp = npool.tile([1, 1024], F32, tag="rp", name=f"rp{j}_{qc}")
            nc.vector.reciprocal_approx_fast(rp[:, :], lp[:, :])
            rpb = npool.tile([1, 1024], BF16, tag="rpb", name=f"rpb{j}_{qc}")
            nc.vector.tensor_copy(rpb[:, :], rp[:, :])
            rpbs[(j, qc)] = rpb

        def attn_norm_tail(j, qc):
            qsl = slice(qc * 512, (qc + 1) * 512)
            rpb = rpbs.pop((j, qc))
            pbc = psm.tile([128, 1024], F32, tag="ps", name=f"pbc{j}_{qc}")
            nc.tensor.matmul(pbc[0:64, 0:512], ones1[0:1, 0:64],
                             rpb[0:1, 0:512], start=True, stop=True)
            nc.tensor.matmul(pbc[64:128, 0:512], ones1[0:1, 0:64],
                             rpb[0:1, 512:1024], start=True, stop=True,
                             tile_position=(0, 64))
            rbc = npool.tile([128, 512], F32, tag="rbc", name=f"rbc{j}_{qc}")
            nc.vector.tensor_copy(rbc[:, :], pbc[:, 0:512])
            nc.vector.tensor_tensor(
                oTv[0:64, j, qsl], oTv[0:64, j, qsl], rbc[0:64, :], MULT)
            nc.vector.tensor_tensor(
                oTv[64:128, j, qsl], oTv[64:128, j, qsl], rbc[64:128, :],
                MULT)

        pending_norm = []

        def flush_norm():
            while pending_norm:
                pending_norm.pop(0)()

        def attn_block(j, qc, interleave=None):
            poA = pso.tile([65, 512], F32, tag="po", name=f"poA{j}_{qc}")
            poB = pso.tile([65, 512], F32, tag="po", name=f"poB{j}_{qc}")
            nsteps = len(interleave) if interleave else 0
            si = 0
            for kt in range(16):
                if interleave and kt % 3 == 0 and si < nsteps:
                    interleave[si]()
                    si += 1
                attn_iter(j, qc, kt, poA, poB)
                if kt == 2:
                    # run the previous block's deferred normalization now:
                    # its DVE chain is long done, so the PE-side broadcast
                    # matmul no longer stalls the engine stream
                    flush_norm()
            while interleave and si < nsteps:
                interleave[si]()
                si += 1
            attn_fast_evac(j, qc, poA, poB)
            pending_norm.append(lambda j=j, qc=qc: attn_norm_tail(j, qc))

        # V proj pipelined with the first attention block (attnV(kt) only
        # needs Vaug[st=kt], which V(st) just produced)
        with tc.tile_pool(name="wv", bufs=1) as wvp:
            wvb = load_w(wv_d, wvp, "wv")

            def v_st(st):
                pv = psm.tile([128, 1024], F32, tag="ps", name=f"pv{st}")
                for nh in range(2):
                    nc.tensor.matmul(
                        pv[:, nh * 512:(nh + 1) * 512], ones1[:, :],
                        bvr[:, nh * 512:(nh + 1) * 512], start=True,
                        stop=False)
                for dt_ in range(8):
                    for nh in range(2):
                        nc.tensor.matmul(
                            pv[:, nh * 512:(nh + 1) * 512],
                            xTv[:, dt_, st * 128:(st + 1) * 128],
                            wvb[dt_][:, nh * 512:(nh + 1) * 512],
                            start=False, stop=(dt_ == 7),
                        )
                dst = Vv[:, st, :, 0:64]
                src_ = pv[:, :].rearrange("p (h d) -> p h d", h=16)
                if st % 2 == 0:
                    nc.scalar.copy(dst, src_)
                else:
                    nc.vector.tensor_copy(dst, src_)

            poA0 = pso.tile([65, 512], F32, tag="po", name="poA0_0")
            poB0 = pso.tile([65, 512], F32, tag="po", name="poB0_0")
            for st in range(16):
                v_st(st)
                attn_iter(0, 0, st, poA0, poB0)
            attn_fast_evac(0, 0, poA0, poB0)
            pending_norm.append(lambda: attn_norm_tail(0, 0))

        for step in kq_chunks(1):
            step()
        for j in range(1, 8):
            attn_block(j, 0,
                       interleave=kq_chunks(j + 1) if j < 7 else None)
        flush_norm()
        wkq_cm.__exit__(None, None, None)
        xTp_cm.__exit__(None, None, None)

        # ---------------- Phase D: proj interleaved with qc=1 attention ---
        with tc.tile_pool(name="wp", bufs=1) as wpp, \
             tc.tile_pool(name="ystg", bufs=2) as ypool:
            wpb = load_w(wp_d, wpp, "wp")

            def proj(qt):
                py = psm.tile([128, 1024], F32, tag="ps", name=f"py{qt}")
                for nh in range(2):
                    nc.tensor.matmul(py[:, nh * 512:(nh + 1) * 512],
                                     ones1[:, :],
                                     bpr[:, nh * 512:(nh + 1) * 512],
                                     start=True, stop=False)
                for j in range(8):
                    for nh in range(2):
                        nc.tensor.matmul(
                            py[:, nh * 512:(nh + 1) * 512],
                            oTv[:, j, qt * 128:(qt + 1) * 128],
                            wpb[j][:, nh * 512:(nh + 1) * 512],
                            start=False, stop=(j == 7),
                        )
                ys = ypool.tile([128, 1024], F32, tag="ys")
                nc.scalar.copy(ys[:, :], py[:, :])
                nc.sync.dma_start(
                    out=out_d[qt * 128:(qt + 1) * 128, :], in_=ys[:, :])

            for j in range(8):
                attn_block(j, 1)
                if j >= 4:
                    proj(j - 4)
            flush_norm()
            for qt in range(4, 8):
                proj(qt)

    nc.finalize()
    return nc


def _in_maps(x, W_qkv, b_qkv, W_proj, b_proj):
    x = np.asarray(x, np.float32)
    W_qkv = np.asarray(W_qkv, np.float32)
    b_qkv = np.asarray(b_qkv, np.float32)
    W_proj = np.ascontiguousarray(np.asarray(W_proj, np.float32))
    b_proj = np.asarray(b_proj, np.float32)
    Wq = np.ascontiguousarray(W_qkv[:, 0:D])
    Wk = np.ascontiguousarray(W_qkv[:, D:2 * D])
    Wv = np.ascontiguousarray(W_qkv[:, 2 * D:3 * D])
    bq, bk, bv = b_qkv[0:D], b_qkv[D:2 * D], b_qkv[2 * D:3 * D]
    bqp = np.ascontiguousarray(bq.reshape(8, 128).T)
    bkp = np.ascontiguousarray(bk.reshape(8, 128).T)
    maps = []
    for c in range(NC_):
        b, qh = c // 2, c % 2
        xb = np.concatenate(
            [x[b, qh * QH:(qh + 1) * QH], x[b, (1 - qh) * QH:(2 - qh) * QH]],
            axis=0)
        maps.append({
            "xb": np.ascontiguousarray(xb), "wq": Wq, "wk": Wk, "wv": Wv,
            "wp": W_proj, "bqp": bqp, "bkp": bkp,
            "bvr": np.ascontiguousarray(bv.reshape(1, D)),
            "bpr": np.ascontiguousarray(b_proj.reshape(1, D)),
        })
    return maps


def run(x, W_qkv, b_qkv, W_proj, b_proj, trace=False, tmpdir=None):
    sys.path.insert(0, "/opt/trn_rl_repo")
    from concourse.bass_utils import run_bass_kernel_spmd

    if "nc" not in _cache:
        _cache["nc"] = _build_nc()
    nc = _cache["nc"]
    maps = _in_maps(x, W_qkv, b_qkv, W_proj, b_proj)
    res = run_bass_kernel_spmd(nc, maps, core_ids=list(range(NC_)),
                               trace=trace, tmpdir=tmpdir)
    y = np.empty((B, S, D), np.float32)
    for c in range(NC_):
        b, qh = c // 2, c % 2
        y[b, qh * QH:(qh + 1) * QH] = res.results[c]["out"]
    return y, res


def kernel(x, W_qkv, b_qkv, W_proj, b_proj):
    y, _ = run(x, W_qkv, b_qkv, W_proj, b_proj, trace=False)
    return y


# revision 26
# speedup vs baseline: 1.2066x; 1.0416x over previous
"""Trainium2 Bass kernel for nn_Attention (B=4, S=2048, D=1024, H=16, hd=64, fp32).

Sharding (zero-communication): 8 cores; core c handles batch b=c//2 and
query-half qh=c%2. Each core computes K,V for its whole batch (all heads),
Q for its query half, attention for all 16 heads over its 1024 queries, and
the output projection for its 1024 rows. The per-core input x is permuted so
the core's query half comes first (softmax over keys is permutation
invariant, so K/V may use the permuted order as long as they agree).

Per-core pipeline (all matmuls bf16, accumulation fp32 in PSUM):
  A. xT[D,S] built via PE transposes of x tiles (bf16).
  B. KT[hd,S] / QT[hd,Sq] (heads stacked 2-per-128-partitions), V[S,hd]
     augmented with a ones column (gives the softmax denominator for free).
     Emission order K0,Q0 -> V -> K1..7,Q1..7 interleaved with attention so
     ScalarE exp work starts as early as possible.
  C. scoresT[k,q] via PE (two heads row-packed with tile_position), exp on
     ScalarE (no max subtraction: |scores/8| < ~3 by construction), attnV
     accumulates (P @ V)^T; the ones column produces l[q]; normalization via
     reciprocal + K=1 broadcast matmuls fused into the PSUM evacuation.
  D. y = outT^T @ W_proj + b_proj (bias via K=1 ones matmul).

One shared [128,1024] PSUM pool (3 slots) + a [65,512] accumulator pool
(2 slots) keeps all phases inside the 8 PSUM banks with fine-grained
slot-level WAR deps instead of phase barriers.
"""

import os
import sys

import numpy as np

B, S, D, H, HD = 4, 2048, 1024, 16, 64
QH = 1024  # queries per core
NC_ = 8

_cache = {}


def _build_nc():
    sys.path.insert(0, "/opt/trn_rl_repo")
    import concourse.bass as bass
    from concourse import bacc
    import concourse.mybir as mybir
    import concourse.tile as tile
    from concourse.masks import make_identity
    from contextlib import ExitStack

    F32 = mybir.dt.float32
    BF16 = mybir.dt.bfloat16
    MULT = mybir.AluOpType.mult
    Exp = mybir.ActivationFunctionType.Exp

    nc = bacc.Bacc()
    x_d = nc.declare_dram_parameter("xb", [S, D], F32, isOutput=False)
    wq_d = nc.declare_dram_parameter("wq", [D, D], F32, isOutput=False)
    wk_d = nc.declare_dram_parameter("wk", [D, D], F32, isOutput=False)
    wv_d = nc.declare_dram_parameter("wv", [D, D], F32, isOutput=False)
    wp_d = nc.declare_dram_parameter("wp", [D, D], F32, isOutput=False)
    bqp_d = nc.declare_dram_parameter("bqp", [128, 8], F32, isOutput=False)
    bkp_d = nc.declare_dram_parameter("bkp", [128, 8], F32, isOutput=False)
    bvr_d = nc.declare_dram_parameter("bvr", [1, D], F32, isOutput=False)
    bpr_d = nc.declare_dram_parameter("bpr", [1, D], F32, isOutput=False)
    out_d = nc.declare_dram_parameter("out", [QH, D], F32, isOutput=True)

    with ExitStack() as ctx:
        tc = ctx.enter_context(tile.TileContext(nc))

        const = ctx.enter_context(tc.tile_pool(name="const", bufs=1))
        ident = const.tile([128, 128], BF16)
        make_identity(nc, ident[:, :])
        ones1 = const.tile([1, 128], BF16)
        nc.vector.memset(ones1[:, :], 1.0)
        bqp = const.tile([128, 8], F32)
        nc.sync.dma_start(out=bqp[:, :], in_=bqp_d[:, :])
        bkp = const.tile([128, 8], F32)
        nc.sync.dma_start(out=bkp[:, :], in_=bkp_d[:, :])
        bvr = const.tile([1, D], BF16)
        nc.gpsimd.dma_start(out=bvr[:, :], in_=bvr_d[:, :])
        bpr = const.tile([1, D], BF16)
        nc.gpsimd.dma_start(out=bpr[:, :], in_=bpr_d[:, :])

        big = ctx.enter_context(tc.tile_pool(name="big", bufs=1))
        KT = big.tile([128, 8 * S], BF16)      # [p(2 heads), (j, k)]
        QT = big.tile([128, 8 * QH], BF16)     # [p(2 heads), (j, q)]
        Vaug = big.tile([128, 16 * 16 * 65], BF16)  # [p(s%128), (st, h, 65)]
        outT = big.tile([128, 8 * QH], BF16)   # [p(2 heads d), (j, q)]

        KTv = KT[:, :].rearrange("p (j k) -> p j k", j=8)
        QTv = QT[:, :].rearrange("p (j q) -> p j q", j=8)
        Vv = Vaug[:, :].rearrange("p (t h e) -> p t h e", t=16, h=16)
        oTv = outT[:, :].rearrange("p (j q) -> p j q", j=8)

        nc.gpsimd.memset(Vaug[:, :], 1.0)

        apool = ctx.enter_context(tc.tile_pool(name="att", bufs=4))
        npool = ctx.enter_context(tc.tile_pool(name="attn", bufs=1))
        xTp_cm = tc.tile_pool(name="xTp", bufs=1)
        xTp = xTp_cm.__enter__()
        xT = xTp.tile([128, 8 * S], BF16)      # [p, (dt, s)]
        xTv = xT[:, :].rearrange("p (d s) -> p d s", d=8)

        # Shared PSUM pools for the whole kernel
        psm = ctx.enter_context(tc.tile_pool(name="psm", bufs=3, space="PSUM"))
        pso = ctx.enter_context(tc.tile_pool(name="pso", bufs=2, space="PSUM"))

        # ---------------- Phase A: xT via PE transposes ----------------
        with tc.tile_pool(name="xstg", bufs=5) as xpool:
            for st in range(16):
                xb16 = xpool.tile([128, D], BF16, tag="xb16")
                nc.gpsimd.dma_start(out=xb16[:, :],
                                    in_=x_d[st * 128:(st + 1) * 128, :])
                pt = psm.tile([128, 1024], BF16, tag="ps", name=f"pt{st}")
                for dt_ in range(8):
                    nc.tensor.transpose(
                        pt[:, dt_ * 128:(dt_ + 1) * 128],
                        xb16[:, dt_ * 128:(dt_ + 1) * 128],
                        ident[:, :],
                    )
                dst = xTv[:, :, st * 128:(st + 1) * 128]
                src = pt[:, :].rearrange("p (d s) -> p d s", d=8)
                if st % 2 == 0:
                    nc.scalar.copy(dst, src)
                else:
                    nc.vector.tensor_copy(dst, src)

        def load_w(wd, pool, tag):
            # SWDGE casts f32 -> bf16 during the DMA
            tiles = []
            for dt_ in range(8):
                wb = pool.tile([128, D], BF16, tag=tag + "b" + str(dt_))
                nc.gpsimd.dma_start(out=wb[:, :],
                                    in_=wd[dt_ * 128:(dt_ + 1) * 128, :])
                tiles.append(wb)
            return tiles

        wkq_cm = tc.tile_pool(name="wkq", bufs=1)
        wkq = wkq_cm.__enter__()
        wkb = load_w(wk_d, wkq, "wk")
        wqb = load_w(wq_d, wkq, "wq")

        def kq_chunks(j):
            # 6 independent emit-steps (4 K s-chunks + 2 Q chunks), each
            # holding one PSUM slot for only ~8 matmuls
            steps = []
            ksteps = []
            for sc in range(4):
                def mk_k(sc=sc):
                    pkc = psm.tile([128, 512], F32, tag="ps",
                                   name=f"pk{j}_{sc}")
                    for dt_ in range(8):
                        nc.tensor.matmul(
                            pkc[:, :],
                            wkb[dt_][:, j * 128:(j + 1) * 128],
                            xTv[:, dt_, sc * 512:(sc + 1) * 512],
                            start=(dt_ == 0), stop=(dt_ == 7),
                        )
                    nc.vector.tensor_scalar_add(
                        KTv[:, j, sc * 512:(sc + 1) * 512], pkc[:, :],
                        bkp[:, j:j + 1])
                ksteps.append(mk_k)
            for qc in range(2):
                def mk_q(qc=qc):
                    pqc = psm.tile([128, 512], F32, tag="ps",
                                   name=f"pq{j}_{qc}")
                    for dt_ in range(8):
                        nc.tensor.matmul(
                            pqc[:, :],
                            wqb[dt_][:, j * 128:(j + 1) * 128],
                            xTv[:, dt_, qc * 512:(qc + 1) * 512],
                            start=(dt_ == 0), stop=(dt_ == 7),
                        )
                    nc.vector.tensor_scalar_add(
                        QTv[:, j, qc * 512:(qc + 1) * 512], pqc[:, :],
                        bqp[:, j:j + 1])
                steps.append(mk_q)
            # K0 then both Q chunks first: unblocks the next block's scores
            # (and the very first exp) as early as possible
            return [ksteps[0]] + steps + ksteps[1:]

        for step in kq_chunks(0):
            step()

        def attn_iter(j, qc, kt, poA, poB):
            qsl = slice(qc * 512, (qc + 1) * 512)
            ps = psm.tile([128, 1024], F32, tag="ps", name=f"ps{j}_{qc}_{kt}")
            nc.tensor.matmul(
                ps[:, 0:512],
                KTv[0:64, j, kt * 128:(kt + 1) * 128],
                QTv[0:64, j, qsl],
                start=True, stop=True, tile_position=(0, 0))
            nc.tensor.matmul(
                ps[:, 512:1024],
                KTv[64:128, j, kt * 128:(kt + 1) * 128],
                QTv[64:128, j, qsl],
                start=True, stop=True, tile_position=(64, 0))
            eP = apool.tile([128, 1024], BF16, tag="eP")
            nc.scalar.activation(eP[:, :], ps[:, :], Exp, scale=0.125)
            nc.tensor.matmul(
                poA[:, :], Vv[:, kt, 2 * j, 0:65], eP[:, 0:512],
                start=(kt == 0), stop=(kt == 15))
            nc.tensor.matmul(
                poB[:, :], Vv[:, kt, 2 * j + 1, 0:65], eP[:, 512:1024],
                start=(kt == 0), stop=(kt == 15))

        rpbs = {}

        def attn_fast_evac(j, qc, poA, poB):
            # unnormalized evacuation frees the PSUM accumulators fast;
            # the reciprocal chain runs on the DVE off the critical path
            qsl = slice(qc * 512, (qc + 1) * 512)
            lp = npool.tile([1, 1024], F32, tag="lp", name=f"lp{j}_{qc}")
            nc.vector.tensor_copy(lp[0:1, 0:512], poA[64:65, :])
            nc.vector.tensor_copy(lp[0:1, 512:1024], poB[64:65, :])
            nc.vector.tensor_copy(oTv[0:64, j, qsl], poA[0:64, :])
            nc.vector.tensor_copy(oTv[64:128, j, qsl], poB[0:64, :])
            rp = npool.tile([1, 1024], F32, tag="rp", name=f"rp{j}_{qc}")
            nc.vector.reciprocal_approx_fast(rp[:, :], lp[:, :])
            rpb = npool.tile([1, 1024], BF16, tag="rpb", name=f"rpb{j}_{qc}")
            nc.vector.tensor_copy(rpb[:, :], rp[:, :])
            rpbs[(j, qc)] = rpb

        def attn_norm_tail(j, qc):
            qsl = slice(qc * 512, (qc + 1) * 512)
            rpb = rpbs.pop((j, qc))
            pbc = psm.tile([128, 1024], F32, tag="ps", name=f"pbc{j}_{qc}")
            nc.tensor.matmul(pbc[0:64, 0:512], ones1[0:1, 0:64],
                             rpb[0:1, 0:512], start=True, stop=True)
            nc.tensor.matmul(pbc[64:128, 0:512], ones1[0:1, 0:64],
                             rpb[0:1, 512:1024], start=True, stop=True,
                             tile_position=(0, 64))
            rbc = npool.tile([128, 512], F32, tag="rbc", name=f"rbc{j}_{qc}")
            nc.vector.tensor_copy(rbc[:, :], pbc[:, 0:512])
            nc.vector.tensor_tensor(
                oTv[0:64, j, qsl], oTv[0:64, j, qsl], rbc[0:64, :], MULT)
            nc.vector.tensor_tensor(
                oTv[64:128, j, qsl], oTv[64:128, j, qsl], rbc[64:128, :],
                MULT)

        pending_norm = []

        def flush_norm():
            while pending_norm:
                pending_norm.pop(0)()

        def attn_group(j, qc, ktg, poA, poB):
            # 2 kt per group: keeps the PE in 64-row tiling mode for 4
            # consecutive score matmuls, then 128-mode for 4 attnV matmuls
            # (mode switches drain the PE array, so alternating per-kt is
            # expensive)
            qsl = slice(qc * 512, (qc + 1) * 512)
            kts = (2 * ktg, 2 * ktg + 1)
            pss = []
            for kt in kts:
                ps = psm.tile([128, 1024], F32, tag="ps",
                              name=f"ps{j}_{qc}_{kt}")
                pss.append(ps)
                nc.tensor.matmul(
                    ps[:, 0:512],
                    KTv[0:64, j, kt * 128:(kt + 1) * 128],
                    QTv[0:64, j, qsl],
                    start=True, stop=True, tile_position=(0, 0))
                nc.tensor.matmul(
                    ps[:, 512:1024],
                    KTv[64:128, j, kt * 128:(kt + 1) * 128],
                    QTv[64:128, j, qsl],
                    start=True, stop=True, tile_position=(64, 0))
            ePs = []
            for ps in pss:
                eP = apool.tile([128, 1024], BF16, tag="eP")
                nc.scalar.activation(eP[:, :], ps[:, :], Exp, scale=0.125)
                ePs.append(eP)
            for kt, eP in zip(kts, ePs):
                nc.tensor.matmul(
                    poA[:, :], Vv[:, kt, 2 * j, 0:65], eP[:, 0:512],
                    start=(kt == 0), stop=(kt == 15))
                nc.tensor.matmul(
                    poB[:, :], Vv[:, kt, 2 * j + 1, 0:65], eP[:, 512:1024],
                    start=(kt == 0), stop=(kt == 15))

        def attn_block(j, qc, interleave=None):
            poA = pso.tile([65, 512], F32, tag="po", name=f"poA{j}_{qc}")
            poB = pso.tile([65, 512], F32, tag="po", name=f"poB{j}_{qc}")
            nsteps = len(interleave) if interleave else 0
            si = 0
            for ktg in range(8):
                attn_group(j, qc, ktg, poA, poB)
                # kq-proj steps are 128-mode; placed right after the
                # 128-mode attnV batch to avoid extra mode switches
                if interleave and si < nsteps and ktg % 2 == 0:
                    interleave[si]()
                    si += 1
                if ktg == 1:
                    flush_norm()
            while interleave and si < nsteps:
                interleave[si]()
                si += 1
            attn_fast_evac(j, qc, poA, poB)
            pending_norm.append(lambda j=j, qc=qc: attn_norm_tail(j, qc))

        # V proj pipelined with the first attention block (attnV(kt) only
        # needs Vaug[st=kt], which V(st) just produced)
        with tc.tile_pool(name="wv", bufs=1) as wvp:
            wvb = load_w(wv_d, wvp, "wv")

            def v_st(st):
                pv = psm.tile([128, 1024], F32, tag="ps", name=f"pv{st}")
                for nh in range(2):
                    nc.tensor.matmul(
                        pv[:, nh * 512:(nh + 1) * 512], ones1[:, :],
                        bvr[:, nh * 512:(nh + 1) * 512], start=True,
                        stop=False)
                for dt_ in range(8):
                    for nh in range(2):
                        nc.tensor.matmul(
                            pv[:, nh * 512:(nh + 1) * 512],
                            xTv[:, dt_, st * 128:(st + 1) * 128],
                            wvb[dt_][:, nh * 512:(nh + 1) * 512],
                            start=False, stop=(dt_ == 7),
                        )
                dst = Vv[:, st, :, 0:64]
                src_ = pv[:, :].rearrange("p (h d) -> p h d", h=16)
                if st % 2 == 0:
                    nc.scalar.copy(dst, src_)
                else:
                    nc.vector.tensor_copy(dst, src_)

            poA0 = pso.tile([65, 512], F32, tag="po", name="poA0_0")
            poB0 = pso.tile([65, 512], F32, tag="po", name="poB0_0")
            for st in range(16):
                v_st(st)
                attn_iter(0, 0, st, poA0, poB0)
            attn_fast_evac(0, 0, poA0, poB0)
            pending_norm.append(lambda: attn_norm_tail(0, 0))

        for step in kq_chunks(1):
            step()
        for j in range(1, 8):
            attn_block(j, 0,
                       interleave=kq_chunks(j + 1) if j < 7 else None)
        flush_norm()
        wkq_cm.__exit__(None, None, None)
        xTp_cm.__exit__(None, None, None)

        # ---------------- Phase D: proj interleaved with qc=1 attention ---
        with tc.tile_pool(name="wp", bufs=1) as wpp, \
             tc.tile_pool(name="ystg", bufs=2) as ypool:
            wpb = load_w(wp_d, wpp, "wp")

            def proj(qt):
                py = psm.tile([128, 1024], F32, tag="ps", name=f"py{qt}")
                for nh in range(2):
                    nc.tensor.matmul(py[:, nh * 512:(nh + 1) * 512],
                                     ones1[:, :],
                                     bpr[:, nh * 512:(nh + 1) * 512],
                                     start=True, stop=False)
                for j in range(8):
                    for nh in range(2):
                        nc.tensor.matmul(
                            py[:, nh * 512:(nh + 1) * 512],
                            oTv[:, j, qt * 128:(qt + 1) * 128],
                            wpb[j][:, nh * 512:(nh + 1) * 512],
                            start=False, stop=(j == 7),
                        )
                ys = ypool.tile([128, 1024], F32, tag="ys")
                nc.scalar.copy(ys[:, :], py[:, :])
                nc.sync.dma_start(
                    out=out_d[qt * 128:(qt + 1) * 128, :], in_=ys[:, :])

            for j in range(8):
                attn_block(j, 1)
                if j >= 4:
                    proj(j - 4)
            flush_norm()
            for qt in range(4, 8):
                proj(qt)

    nc.finalize()
    return nc


def _in_maps(x, W_qkv, b_qkv, W_proj, b_proj):
    x = np.asarray(x, np.float32)
    W_qkv = np.asarray(W_qkv, np.float32)
    b_qkv = np.asarray(b_qkv, np.float32)
    W_proj = np.ascontiguousarray(np.asarray(W_proj, np.float32))
    b_proj = np.asarray(b_proj, np.float32)
    Wq = np.ascontiguousarray(W_qkv[:, 0:D])
    Wk = np.ascontiguousarray(W_qkv[:, D:2 * D])
    Wv = np.ascontiguousarray(W_qkv[:, 2 * D:3 * D])
    bq, bk, bv = b_qkv[0:D], b_qkv[D:2 * D], b_qkv[2 * D:3 * D]
    bqp = np.ascontiguousarray(bq.reshape(8, 128).T)
    bkp = np.ascontiguousarray(bk.reshape(8, 128).T)
    maps = []
    for c in range(NC_):
        b, qh = c // 2, c % 2
        xb = np.concatenate(
            [x[b, qh * QH:(qh + 1) * QH], x[b, (1 - qh) * QH:(2 - qh) * QH]],
            axis=0)
        maps.append({
            "xb": np.ascontiguousarray(xb), "wq": Wq, "wk": Wk, "wv": Wv,
            "wp": W_proj, "bqp": bqp, "bkp": bkp,
            "bvr": np.ascontiguousarray(bv.reshape(1, D)),
            "bpr": np.ascontiguousarray(b_proj.reshape(1, D)),
        })
    return maps


def run(x, W_qkv, b_qkv, W_proj, b_proj, trace=False, tmpdir=None):
    sys.path.insert(0, "/opt/trn_rl_repo")
    from concourse.bass_utils import run_bass_kernel_spmd

    if "nc" not in _cache:
        _cache["nc"] = _build_nc()
    nc = _cache["nc"]
    maps = _in_maps(x, W_qkv, b_qkv, W_proj, b_proj)
    res = run_bass_kernel_spmd(nc, maps, core_ids=list(range(NC_)),
                               trace=trace, tmpdir=tmpdir)
    y = np.empty((B, S, D), np.float32)
    for c in range(NC_):
        b, qh = c // 2, c % 2
        y[b, qh * QH:(qh + 1) * QH] = res.results[c]["out"]
    return y, res


def kernel(x, W_qkv, b_qkv, W_proj, b_proj):
    y, _ = run(x, W_qkv, b_qkv, W_proj, b_proj, trace=False)
    return y
